# revision 28
# baseline (speedup 1.0000x reference)
"""Fused LayerNorm + multi-head attention + out-projection for Trainium2.

Problem: x[2,2048,1024] -> LN -> QKV (16 heads, dh=64) -> softmax attention
-> out proj.  Sharded over 8 NeuronCores as batch(2) x head-groups(4)
(Megatron tensor parallel): each core handles one batch entry and 4 heads,
computing a partial out-projection; the host sums the 4 partials per batch.

Per-core dataflow (T=2048 tokens, D=1024, 4 local heads, dh=64), bf16
matmul inputs everywhere (fp32 psum accumulation):
  A) per 128-token tile: LN via bn_stats/bn_aggr (fp32), xn cast to bf16,
     PE-transpose to persistent xnT [d, tok].
  B) v natural [tok, c] for all 4 heads and qT/kT [c, tok] for head pair 0.
  C) per (head-pair, i-chunk 512): software-pipelined over j-tiles:
     S^T[j,i] paired matmul (two heads on disjoint 64-row PE groups);
     exp((1/8)S) on ACT -> bf16; O[i, d|r] += ex_slice.T @ (V|1) as M=128,
     N=66 bf16 matmuls accumulating in PSUM (denominator r in column 64).
     The scalar-engine exp stream is the wall-clock floor; leftover PE
     work (pair-1 q/k projections during pr=0, O-transposes + output
     projections during pr=1) is dripped one instruction per j-step into
     the PE idle gaps so the tensor engine stays busy enough to hold the
     HAM clock at 2.4 GHz and never stalls the exp stream.
  Normalization is a per-partition scalar multiply (1/r) on DVE in [i, d]
  layout, then PE-transpose back to OT [c, i] for the out-projection.
gamma is folded into w_qkv on the host; beta/b_out are zeros by spec
(b_out still added on the host).
"""
import numpy as np

import concourse.bacc as bacc
import concourse.mybir as mybir
import concourse.tile as tile
from concourse import bass_utils
from concourse.bass import broadcast_tensor_aps
from concourse.masks import make_identity

F32 = mybir.dt.float32
BF16 = mybir.dt.bfloat16
AF = mybir.ActivationFunctionType
ALU = mybir.AluOpType

T = 2048          # tokens per core (one batch entry)
D = 1024          # model dim
HL = 4            # local heads per core
DH = 64           # head dim
CI = HL * DH      # local inner dim = 256
NT = T // 128     # 16 token tiles
NK = D // 128     # 8 dim chunks
LN_EPS = 1e-5
SCALE = DH ** -0.5

_NC_CACHE = {}


def _build():
    nc = bacc.Bacc("TRN2", target_bir_lowering=False, debug=False)

    x = nc.dram_tensor("x", [T, D], F32, kind="ExternalInput")
    wq = nc.dram_tensor("wq", [D, CI], BF16, kind="ExternalInput")
    wk = nc.dram_tensor("wk", [D, CI], BF16, kind="ExternalInput")
    wv = nc.dram_tensor("wv", [D, CI], BF16, kind="ExternalInput")
    wo = nc.dram_tensor("wo", [CI, D], BF16, kind="ExternalInput")
    out = nc.dram_tensor("out", [T, D], F32, kind="ExternalOutput")

    x_t = x.rearrange("(t p) d -> t p d", p=128)          # [16, 128, 1024]
    out_t = out.rearrange("(t p) d -> t p d", p=128)
    wq_t = wq.rearrange("(c p) n -> p c n", p=128)        # [128, 8, 256]
    wk_t = wk.rearrange("(c p) n -> p c n", p=128)
    wv_t = wv.rearrange("(c p) n -> p c n", p=128)
    wo_t = wo.rearrange("(c p) n -> p c n", p=128)        # [128, 2, 1024]

    with tile.TileContext(nc) as tc:
        with tc.tile_pool(name="persist", bufs=1) as persist:
            # constants
            eps = persist.tile([128, 1], F32, name="eps")
            nc.vector.memset(eps, LN_EPS)
            ident_f = persist.tile([128, 128], F32, name="ident_f")
            make_identity(nc, ident_f)
            ident = persist.tile([128, 128], BF16, name="ident")
            nc.vector.tensor_copy(out=ident, in_=ident_f)

            # persistent activations / weights (all bf16)
            xnT = persist.tile([128, NK, T], BF16, name="xnT")  # 32KB/p
            qT = persist.tile([128, 2, T], BF16, name="qT")
            kT = persist.tile([128, 2, T], BF16, name="kT")
            vext = persist.tile([128, NT, HL, 66], BF16, name="vext")
            OT = persist.tile([128, 2, T], BF16, name="OT")
            o_nat = persist.tile([128, 4, 4, HL, 64], BF16, name="o_nat")
            wq_s = persist.tile([128, NK, CI], BF16, name="wq_s")
            wk_s = persist.tile([128, NK, CI], BF16, name="wk_s")
            wv_s = persist.tile([128, NK, CI], BF16, name="wv_s")
            wo_s = persist.tile([128, 2, D], BF16, name="wo_s")

            # ones column (64) of vext; column 65 is zero padding
            nc.vector.memset(vext[:, :, :, 64:65], 1.0)
            nc.vector.memset(vext[:, :, :, 65:66], 0.0)

            # ---- Phase A: LN + transpose; B0: v (all heads) + q/k pair 0
            with (
                tc.tile_pool(name="ab_sb", bufs=3) as ab_sb,
                tc.tile_pool(name="ab_sm", bufs=8) as ab_sm,
                tc.tile_pool(name="ab_ps", bufs=1, space="PSUM") as ab_ps,
            ):
                xts, xns = {}, {}

                def ln_dma(tt):
                    xt = ab_sb.tile([128, D], F32, tag="xt", name="xt", bufs=4)
                    (nc.sync if tt % 2 == 0 else nc.scalar).dma_start(
                        xt, x_t[tt])
                    xts[tt] = xt

                def ln_compute(tt):
                    # stats (DVE) -> rstd, -mu*rstd -> xn on ACT (Identity is
                    # in the sqrt table set, so no extra table load)
                    xt = xts.pop(tt)
                    stats = ab_sm.tile([128, 2, 6], F32, tag="stats",
                                       name="stats")
                    xr = xt.rearrange("p (c f) -> p c f", f=512)
                    for c in range(2):
                        nc.vector.bn_stats(out=stats[:, c, :], in_=xr[:, c, :])
                    mv = ab_sm.tile([128, 2], F32, tag="mv", name="mv")
                    nc.vector.bn_aggr(out=mv, in_=stats)
                    rstd = ab_sm.tile([128, 1], F32, tag="rstd", name="rstd")
                    nc.scalar.activation(out=rstd, in_=mv[:, 1:2], func=AF.Sqrt,
                                         bias=eps, scale=1.0)
                    nc.vector.reciprocal(out=rstd, in_=rstd)
                    nmr = ab_sm.tile([128, 1], F32, tag="nmr", name="nmr")
                    nc.vector.tensor_scalar(out=nmr, in0=mv[:, 0:1],
                                            scalar1=rstd, scalar2=-1.0,
                                            op0=ALU.mult, op1=ALU.mult)
                    xn = ab_sb.tile([128, D], BF16, tag="xn", name="xn")
                    nc.scalar.activation(out=xn, in_=xt, func=AF.Identity,
                                         bias=nmr, scale=rstd)
                    xns[tt] = xn

                def t_tile(tt):
                    # PE-transpose one token tile into xnT
                    xn = xns.pop(tt)
                    for kc4 in range(2):
                        pt = ab_ps.tile([128, 4, 128], BF16, tag="tp",
                                        name="pt", bufs=2)
                        for q in range(4):
                            kc = kc4 * 4 + q
                            nc.tensor.transpose(
                                pt[:, q, :], xn[:, kc * 128:(kc + 1) * 128],
                                ident)
                        nc.vector.tensor_copy(
                            out=xnT[:, kc4 * 4:kc4 * 4 + 4,
                                    tt * 128:(tt + 1) * 128], in_=pt)

                def v_tile(tt):
                    # v natural (all 4 heads) for one token tile
                    pv = ab_ps.tile([128, CI], F32, tag="pv", name="pv",
                                    bufs=2)
                    for kc in range(NK):
                        nc.tensor.matmul(
                            pv,
                            lhsT=xnT[:, kc, tt * 128:(tt + 1) * 128],
                            rhs=wv_s[:, kc, :],
                            start=(kc == 0), stop=(kc == NK - 1))
                    nc.scalar.copy(
                        out=vext[:, tt, :, 0:64],
                        in_=pv.rearrange("p (h d) -> p h d", h=HL))

                def qk_chunk(ic):
                    # qT/kT pair 0 for one token chunk
                    isl = slice(ic * 512, (ic + 1) * 512)
                    pq = ab_ps.tile([128, 2, 512], F32, tag="pq", name="pq",
                                    bufs=2)
                    for kc in range(NK):
                        for pc in range(2):
                            w_src = wq_s if pc == 0 else wk_s
                            nc.tensor.matmul(
                                pq[:, pc, :],
                                lhsT=w_src[:, kc, 0:128],
                                rhs=xnT[:, kc, isl],
                                start=(kc == 0), stop=(kc == NK - 1))
                    nc.scalar.copy(out=qT[:, 0, isl], in_=pq[:, 0, :])
                    nc.scalar.copy(out=kT[:, 0, isl], in_=pq[:, 1, :])

                # staged software pipeline: x DMA runs 6 tiles ahead, the
                # LN chain 2 ahead, the transpose 1 ahead of the v/qk
                # projections, so every engine queue stays fed
                for tt in range(3):
                    ln_dma(tt)
                nc.sync.dma_start(wq_s, wq_t)
                nc.sync.dma_start(wk_s, wk_t)
                nc.sync.dma_start(wv_s, wv_t)
                nc.sync.dma_start(wo_s, wo_t)
                ln_compute(0)
                ln_compute(1)
                t_tile(0)
                for tt in range(NT):
                    v_tile(tt)
                    if tt % 4 == 3:
                        qk_chunk(tt // 4)
                    if tt + 3 < NT:
                        ln_dma(tt + 3)
                    if tt + 2 < NT:
                        ln_compute(tt + 2)
                    if tt + 1 < NT:
                        t_tile(tt + 1)

            # ---------------- Phase C: attention ----------------
            with (
                tc.tile_pool(name="c_sb", bufs=1) as c_sb,
                tc.tile_pool(name="c_ps", bufs=1, space="PSUM") as c_ps,
            ):
                def d_one(tt):
                    # one out-projection token tile
                    pd = c_ps.tile([128, 1024], F32, tag="s", name="pd",
                                   bufs=2)
                    for ncn in range(2):
                        for ck in range(2):
                            nc.tensor.matmul(
                                pd[:, ncn * 512:(ncn + 1) * 512],
                                lhsT=OT[:, ck, tt * 128:(tt + 1) * 128],
                                rhs=wo_s[:, ck, ncn * 512:(ncn + 1) * 512],
                                start=(ck == 0), stop=(ck == 1))
                    ot_sb = c_sb.tile([128, 1024], F32, tag="ot", name="ot_sb",
                                      bufs=3)
                    nc.vector.tensor_copy(out=ot_sb, in_=pd)
                    nc.sync.dma_start(out_t[tt], ot_sb)

                def t_one(ic, isub):
                    # transpose normalized O [i, c] -> OT [c, i] for one i-sub
                    tp2 = c_ps.tile([128, 2, 128], BF16, tag="aux",
                                    name="tp2", bufs=2)
                    for ck in range(2):
                        nc.tensor.transpose(
                            tp2[:, ck, :],
                            o_nat[:, ic, isub, 2 * ck:2 * ck + 2, :]
                            .rearrange("p a b -> p (a b)"),
                            ident)
                    nc.vector.tensor_copy(
                        out=OT[:, :, ic * 512 + isub * 128:
                               ic * 512 + (isub + 1) * 128],
                        in_=tp2)

                for pr in range(2):
                    for ic in range(4):
                        isl = slice(ic * 512, (ic + 1) * 512)
                        # drip-feed schedule: one small PE task per j-step,
                        # keeping the tensor engine busy under the exp stream
                        drip = {}
                        if pr == 0:
                            # pair-1 q/k projections for this token chunk;
                            # q and k each use a 1-bank aux psum allocation
                            pq1 = {}

                            def qk1(kc, pc, _isl=isl):
                                if kc == 0:
                                    pq1[pc] = c_ps.tile([128, 512], F32,
                                                        tag="aux", name="pq1",
                                                        bufs=2)
                                w_src = wq_s if pc == 0 else wk_s
                                nc.tensor.matmul(
                                    pq1[pc],
                                    lhsT=w_src[:, kc, 128:256],
                                    rhs=xnT[:, kc, _isl],
                                    start=(kc == 0), stop=(kc == NK - 1))

                            def qk1_copy(pc, _isl=isl):
                                dst = qT if pc == 0 else kT
                                nc.vector.tensor_copy(out=dst[:, 1, _isl],
                                                      in_=pq1.pop(pc))

                            for kc in range(NK):
                                drip[kc] = (qk1, (kc, 0))
                                drip[NK + kc] = (qk1, (kc, 1))
                            drip[NK + NK] = (qk1_copy, (0,))
                        else:
                            # epilogue of chunk ic-1: transposes + projection
                            if ic > 0:
                                for i in range(4):
                                    drip[1 + 2 * i] = (t_one, (ic - 1, i))
                                    drip[2 + 2 * i] = (d_one, ((ic - 1) * 4 + i,))

                        po = [c_ps.tile([128, 4, 128], F32, tag=f"o{j}",
                                        name=f"po{j}", bufs=1)
                              for j in range(2)]
                        nc.vector.memset(po[0], 0.0)
                        nc.vector.memset(po[1], 0.0)

                        def o_one(jt, exv):
                            for hp in range(2):
                                for isub in range(4):
                                    nc.tensor.matmul(
                                        po[hp][:, isub, 0:66],
                                        lhsT=exv[:, hp * 512 + isub * 128:
                                                 hp * 512 + (isub + 1) * 128],
                                        rhs=vext[:, jt, pr * 2 + hp, :],
                                        start=False, stop=(jt == NT - 1),
                                        skip_group_check=True)

                        # software-pipelined: issue S(jt)+exp(jt) three steps
                        # ahead of O(jt-3) so the in-order PE queue never
                        # blocks the exp stream behind the O accumulation's
                        # chunk-boundary dependency (normalize + re-zero)
                        DEPTH = 1
                        exs = {}
                        for jt in range(NT + DEPTH):
                            if jt < NT:
                                ps_s = c_ps.tile([128, 1024], F32, tag="s",
                                                 name="ps_s", bufs=2)
                                for hp in range(2):
                                    po64 = hp * 64
                                    nc.tensor.matmul(
                                        ps_s[:, hp * 512:(hp + 1) * 512],
                                        lhsT=kT[po64:po64 + 64, pr,
                                                jt * 128:(jt + 1) * 128],
                                        rhs=qT[po64:po64 + 64, pr, isl],
                                        start=True, stop=True)
                                ex = c_sb.tile([128, 1024], BF16, tag="ex",
                                               name="ex", bufs=8)
                                nc.scalar.activation(out=ex, in_=ps_s,
                                                     func=AF.Exp, scale=SCALE)
                                exs[jt] = ex
                            if jt >= DEPTH:
                                o_one(jt - DEPTH, exs.pop(jt - DEPTH))
                            if jt in drip:
                                fn, args = drip.pop(jt)
                                fn(*args)
                        # pair-1 k copy (DVE) after its accumulation
                        if pr == 0:
                            qk1_copy(1)
                        # normalize: batched per-(partition, i-sub) scalar
                        # 1/r via a 0-stride broadcast tensor_tensor
                        for hp in range(2):
                            rcp4 = c_sb.tile([128, 4, 1], F32, tag="rcp",
                                             name="rcp4", bufs=4)
                            nc.vector.reciprocal(out=rcp4,
                                                 in_=po[hp][:, :, 64:65])
                            dst = o_nat[:, ic, :, pr * 2 + hp, :]
                            src = po[hp][:, :, 0:64]
                            rb, _ = broadcast_tensor_aps(rcp4[:, :, :], src)
                            nc.vector.tensor_tensor(out=dst, in0=src, in1=rb,
                                                    op=ALU.mult)
                # tail: epilogue of the last chunk
                for i in range(4):
                    t_one(3, i)
                for i in range(4):
                    d_one(12 + i)

    nc.compile()
    return nc


def kernel(x, gamma, beta, w_qkv, w_out, b_out):
    """Full inputs in, full output out.  Shards batch x head-groups over 8
    cores, runs the SPMD Bass kernel, and sums the partial projections."""
    import ml_dtypes
    bf16 = ml_dtypes.bfloat16

    if "nc" not in _NC_CACHE:
        _NC_CACHE["nc"] = _build()
    nc = _NC_CACHE["nc"]

    x = np.asarray(x, dtype=np.float32)
    gamma = np.asarray(gamma, dtype=np.float32)
    w_qkv = np.asarray(w_qkv, dtype=np.float32)
    w_out = np.asarray(w_out, dtype=np.float32)
    b_out = np.asarray(b_out, dtype=np.float32)

    wg = w_qkv * gamma[:, None]  # fold LN gamma into the QKV projection
    in_maps = []
    for core in range(8):
        b, g = core // 4, core % 4
        cs = slice(g * CI, (g + 1) * CI)
        in_maps.append({
            "x": np.ascontiguousarray(x[b]),
            "wq": np.ascontiguousarray(wg[:, 0 * 1024:1 * 1024][:, cs]).astype(bf16),
            "wk": np.ascontiguousarray(wg[:, 1 * 1024:2 * 1024][:, cs]).astype(bf16),
            "wv": np.ascontiguousarray(wg[:, 2 * 1024:3 * 1024][:, cs]).astype(bf16),
            "wo": np.ascontiguousarray(w_out[cs, :]).astype(bf16),
        })

    res = bass_utils.run_bass_kernel_spmd(nc, in_maps, core_ids=list(range(8)))
    parts = [r["out"] for r in res.results]
    full = np.stack([
        parts[0] + parts[1] + parts[2] + parts[3],
        parts[4] + parts[5] + parts[6] + parts[7],
    ]).astype(np.float32)
    return full + b_out


# revision 29
# speedup vs baseline: 1.0030x; 1.0030x over previous
"""Fused LayerNorm + multi-head attention + out-projection for Trainium2.

Problem: x[2,2048,1024] -> LN -> QKV (16 heads, dh=64) -> softmax attention
-> out proj.  Sharded over 8 NeuronCores as batch(2) x head-groups(4)
(Megatron tensor parallel): each core handles one batch entry and 4 heads,
computing a partial out-projection; the host sums the 4 partials per batch.

Per-core dataflow (T=2048 tokens, D=1024, 4 local heads, dh=64), bf16
matmul inputs everywhere (fp32 psum accumulation):
  A) per 128-token tile: LN via bn_stats/bn_aggr (fp32), xn cast to bf16,
     PE-transpose to persistent xnT [d, tok].
  B) v natural [tok, c] for all 4 heads and qT/kT [c, tok] for head pair 0.
  C) per (head-pair, i-chunk 512): software-pipelined over j-tiles:
     S^T[j,i] paired matmul (two heads on disjoint 64-row PE groups);
     exp((1/8)S) on ACT -> bf16; O[i, d|r] += ex_slice.T @ (V|1) as M=128,
     N=66 bf16 matmuls accumulating in PSUM (denominator r in column 64).
     The scalar-engine exp stream is the wall-clock floor; leftover PE
     work (pair-1 q/k projections during pr=0, O-transposes + output
     projections during pr=1) is dripped one instruction per j-step into
     the PE idle gaps so the tensor engine stays busy enough to hold the
     HAM clock at 2.4 GHz and never stalls the exp stream.
  Normalization is a per-partition scalar multiply (1/r) on DVE in [i, d]
  layout, then PE-transpose back to OT [c, i] for the out-projection.
gamma is folded into w_qkv on the host; beta/b_out are zeros by spec
(b_out still added on the host).
"""
import numpy as np

import concourse.bacc as bacc
import concourse.mybir as mybir
import concourse.tile as tile
from concourse import bass_utils
from concourse.bass import broadcast_tensor_aps
from concourse.masks import make_identity

F32 = mybir.dt.float32
BF16 = mybir.dt.bfloat16
AF = mybir.ActivationFunctionType
ALU = mybir.AluOpType

T = 2048          # tokens per core (one batch entry)
D = 1024          # model dim
HL = 4            # local heads per core
DH = 64           # head dim
CI = HL * DH      # local inner dim = 256
NT = T // 128     # 16 token tiles
NK = D // 128     # 8 dim chunks
LN_EPS = 1e-5
SCALE = DH ** -0.5

_NC_CACHE = {}


def _build():
    nc = bacc.Bacc("TRN2", target_bir_lowering=False, debug=False)

    x = nc.dram_tensor("x", [T, D], F32, kind="ExternalInput")
    wq = nc.dram_tensor("wq", [D, CI], BF16, kind="ExternalInput")
    wk = nc.dram_tensor("wk", [D, CI], BF16, kind="ExternalInput")
    wv = nc.dram_tensor("wv", [D, CI], BF16, kind="ExternalInput")
    wo = nc.dram_tensor("wo", [CI, D], BF16, kind="ExternalInput")
    out = nc.dram_tensor("out", [T, D], F32, kind="ExternalOutput")

    x_t = x.rearrange("(t p) d -> t p d", p=128)          # [16, 128, 1024]
    out_t = out.rearrange("(t p) d -> t p d", p=128)
    wq_t = wq.rearrange("(c p) n -> p c n", p=128)        # [128, 8, 256]
    wk_t = wk.rearrange("(c p) n -> p c n", p=128)
    wv_t = wv.rearrange("(c p) n -> p c n", p=128)
    wo_t = wo.rearrange("(c p) n -> p c n", p=128)        # [128, 2, 1024]

    with tile.TileContext(nc) as tc:
        with tc.tile_pool(name="persist", bufs=1) as persist:
            # constants
            eps = persist.tile([128, 1], F32, name="eps")
            nc.vector.memset(eps, LN_EPS)
            ident_f = persist.tile([128, 128], F32, name="ident_f")
            make_identity(nc, ident_f)
            ident = persist.tile([128, 128], BF16, name="ident")
            nc.vector.tensor_copy(out=ident, in_=ident_f)

            # persistent activations / weights (all bf16)
            xnT = persist.tile([128, NK, T], BF16, name="xnT")  # 32KB/p
            qT = persist.tile([128, 2, T], BF16, name="qT")
            kT = persist.tile([128, 2, T], BF16, name="kT")
            vext = persist.tile([128, NT, HL, 66], BF16, name="vext")
            OT = persist.tile([128, 2, T], BF16, name="OT")
            o_nat = persist.tile([128, 4, 4, HL, 64], BF16, name="o_nat")
            wq_s = persist.tile([128, NK, CI], BF16, name="wq_s")
            wk_s = persist.tile([128, NK, CI], BF16, name="wk_s")
            wv_s = persist.tile([128, NK, CI], BF16, name="wv_s")
            wo_s = persist.tile([128, 2, D], BF16, name="wo_s")

            # ones column (64) of vext; column 65 is zero padding
            nc.vector.memset(vext[:, :, :, 64:65], 1.0)
            nc.vector.memset(vext[:, :, :, 65:66], 0.0)

            # ---- Phase A: LN + transpose; B0: v (all heads) + q/k pair 0
            with (
                tc.tile_pool(name="ab_sb", bufs=3) as ab_sb,
                tc.tile_pool(name="ab_sm", bufs=8) as ab_sm,
                tc.tile_pool(name="ab_ps", bufs=1, space="PSUM") as ab_ps,
            ):
                xts, xns = {}, {}

                def ln_dma(tt):
                    xt = ab_sb.tile([128, D], F32, tag="xt", name="xt", bufs=4)
                    (nc.sync if tt % 2 == 0 else nc.scalar).dma_start(
                        xt, x_t[tt])
                    xts[tt] = xt

                def ln_compute(tt):
                    # stats (DVE) -> rstd, -mu*rstd -> xn on ACT (Identity is
                    # in the sqrt table set, so no extra table load)
                    xt = xts.pop(tt)
                    stats = ab_sm.tile([128, 2, 6], F32, tag="stats",
                                       name="stats")
                    xr = xt.rearrange("p (c f) -> p c f", f=512)
                    for c in range(2):
                        nc.vector.bn_stats(out=stats[:, c, :], in_=xr[:, c, :])
                    mv = ab_sm.tile([128, 2], F32, tag="mv", name="mv")
                    nc.vector.bn_aggr(out=mv, in_=stats)
                    rstd = ab_sm.tile([128, 1], F32, tag="rstd", name="rstd")
                    nc.scalar.activation(out=rstd, in_=mv[:, 1:2], func=AF.Sqrt,
                                         bias=eps, scale=1.0)
                    nc.vector.reciprocal(out=rstd, in_=rstd)
                    nmr = ab_sm.tile([128, 1], F32, tag="nmr", name="nmr")
                    nc.vector.tensor_scalar(out=nmr, in0=mv[:, 0:1],
                                            scalar1=rstd, scalar2=-1.0,
                                            op0=ALU.mult, op1=ALU.mult)
                    xn = ab_sb.tile([128, D], BF16, tag="xn", name="xn")
                    nc.scalar.activation(out=xn, in_=xt, func=AF.Identity,
                                         bias=nmr, scale=rstd)
                    xns[tt] = xn

                def t_tile(tt):
                    # PE-transpose one token tile into xnT
                    xn = xns.pop(tt)
                    for kc4 in range(2):
                        pt = ab_ps.tile([128, 4, 128], BF16, tag="tp",
                                        name="pt", bufs=2)
                        for q in range(4):
                            kc = kc4 * 4 + q
                            nc.tensor.transpose(
                                pt[:, q, :], xn[:, kc * 128:(kc + 1) * 128],
                                ident)
                        nc.vector.tensor_copy(
                            out=xnT[:, kc4 * 4:kc4 * 4 + 4,
                                    tt * 128:(tt + 1) * 128], in_=pt)

                def v_tile(tt):
                    # v natural (all 4 heads) for one token tile
                    pv = ab_ps.tile([128, CI], F32, tag="pv", name="pv",
                                    bufs=2)
                    for kc in range(NK):
                        nc.tensor.matmul(
                            pv,
                            lhsT=xnT[:, kc, tt * 128:(tt + 1) * 128],
                            rhs=wv_s[:, kc, :],
                            start=(kc == 0), stop=(kc == NK - 1))
                    nc.scalar.copy(
                        out=vext[:, tt, :, 0:64],
                        in_=pv.rearrange("p (h d) -> p h d", h=HL))

                def qk_chunk(ic):
                    # qT/kT pair 0 for one token chunk
                    isl = slice(ic * 512, (ic + 1) * 512)
                    pq = ab_ps.tile([128, 2, 512], F32, tag="pq", name="pq",
                                    bufs=2)
                    for kc in range(NK):
                        for pc in range(2):
                            w_src = wq_s if pc == 0 else wk_s
                            nc.tensor.matmul(
                                pq[:, pc, :],
                                lhsT=w_src[:, kc, 0:128],
                                rhs=xnT[:, kc, isl],
                                start=(kc == 0), stop=(kc == NK - 1))
                    nc.scalar.copy(out=qT[:, 0, isl], in_=pq[:, 0, :])
                    nc.scalar.copy(out=kT[:, 0, isl], in_=pq[:, 1, :])

                # staged software pipeline: x DMA runs 6 tiles ahead, the
                # LN chain 2 ahead, the transpose 1 ahead of the v/qk
                # projections, so every engine queue stays fed
                for tt in range(3):
                    ln_dma(tt)
                nc.sync.dma_start(wq_s, wq_t)
                nc.sync.dma_start(wk_s, wk_t)
                nc.sync.dma_start(wv_s, wv_t)
                nc.sync.dma_start(wo_s, wo_t)
                ln_compute(0)
                ln_compute(1)
                t_tile(0)
                for tt in range(NT):
                    v_tile(tt)
                    if tt % 4 == 3:
                        qk_chunk(tt // 4)
                    if tt + 3 < NT:
                        ln_dma(tt + 3)
                    if tt + 2 < NT:
                        ln_compute(tt + 2)
                    if tt + 1 < NT:
                        t_tile(tt + 1)

            # ---------------- Phase C: attention ----------------
            with (
                tc.tile_pool(name="c_sb", bufs=1) as c_sb,
                tc.tile_pool(name="c_ps", bufs=1, space="PSUM") as c_ps,
            ):
                def d_one(tt):
                    # one out-projection token tile
                    pd = c_ps.tile([128, 1024], F32, tag="s", name="pd",
                                   bufs=2)
                    for ncn in range(2):
                        for ck in range(2):
                            nc.tensor.matmul(
                                pd[:, ncn * 512:(ncn + 1) * 512],
                                lhsT=OT[:, ck, tt * 128:(tt + 1) * 128],
                                rhs=wo_s[:, ck, ncn * 512:(ncn + 1) * 512],
                                start=(ck == 0), stop=(ck == 1))
                    ot_sb = c_sb.tile([128, 1024], F32, tag="ot", name="ot_sb",
                                      bufs=3)
                    nc.vector.tensor_copy(out=ot_sb, in_=pd)
                    nc.sync.dma_start(out_t[tt], ot_sb)

                def t_one(ic, isub):
                    # transpose normalized O [i, c] -> OT [c, i] for one i-sub
                    tp2 = c_ps.tile([128, 2, 128], BF16, tag="aux",
                                    name="tp2", bufs=2)
                    for ck in range(2):
                        nc.tensor.transpose(
                            tp2[:, ck, :],
                            o_nat[:, ic, isub, 2 * ck:2 * ck + 2, :]
                            .rearrange("p a b -> p (a b)"),
                            ident)
                    nc.vector.tensor_copy(
                        out=OT[:, :, ic * 512 + isub * 128:
                               ic * 512 + (isub + 1) * 128],
                        in_=tp2)

                norm_q = []  # deferred per-hp normalizations
                for pr in range(2):
                    for ic in range(4):
                        isl = slice(ic * 512, (ic + 1) * 512)
                        # drip-feed schedule: one small PE task per j-step,
                        # keeping the tensor engine busy under the exp stream
                        drip = {}
                        if pr == 0:
                            # pair-1 q/k projections for this token chunk;
                            # q and k each use a 1-bank aux psum allocation
                            pq1 = {}

                            def qk1(kc, pc, _isl=isl):
                                if kc == 0:
                                    pq1[pc] = c_ps.tile([128, 512], F32,
                                                        tag="aux", name="pq1",
                                                        bufs=2)
                                w_src = wq_s if pc == 0 else wk_s
                                nc.tensor.matmul(
                                    pq1[pc],
                                    lhsT=w_src[:, kc, 128:256],
                                    rhs=xnT[:, kc, _isl],
                                    start=(kc == 0), stop=(kc == NK - 1))

                            def qk1_copy(pc, _isl=isl):
                                dst = qT if pc == 0 else kT
                                nc.vector.tensor_copy(out=dst[:, 1, _isl],
                                                      in_=pq1.pop(pc))

                            for kc in range(NK):
                                drip[kc] = (qk1, (kc, 0))
                                drip[NK + kc] = (qk1, (kc, 1))
                            drip[NK + NK] = (qk1_copy, (0,))
                        else:
                            # epilogue of chunk ic-1: transposes + projection
                            if ic > 0:
                                for i in range(4):
                                    drip[1 + 2 * i] = (t_one, (ic - 1, i))
                                    drip[2 + 2 * i] = (d_one, ((ic - 1) * 4 + i,))

                        po = [c_ps.tile([128, 4, 128], F32, tag=f"o{j}",
                                        name=f"po{j}", bufs=1)
                              for j in range(2)]
                        if norm_q:
                            norm_q.pop(0)()     # norm prev hp0 (frees o0)
                        nc.vector.memset(po[0], 0.0)
                        if norm_q:
                            norm_q.pop(0)()     # norm prev hp1 (frees o1)
                        nc.vector.memset(po[1], 0.0)

                        def o_one(jt, exv):
                            for hp in range(2):
                                for isub in range(4):
                                    nc.tensor.matmul(
                                        po[hp][:, isub, 0:66],
                                        lhsT=exv[:, hp * 512 + isub * 128:
                                                 hp * 512 + (isub + 1) * 128],
                                        rhs=vext[:, jt, pr * 2 + hp, :],
                                        start=False, stop=(jt == NT - 1),
                                        skip_group_check=True)

                        # software-pipelined: issue S(jt)+exp(jt) three steps
                        # ahead of O(jt-3) so the in-order PE queue never
                        # blocks the exp stream behind the O accumulation's
                        # chunk-boundary dependency (normalize + re-zero)
                        DEPTH = 1
                        exs = {}
                        for jt in range(NT + DEPTH):
                            if jt < NT:
                                ps_s = c_ps.tile([128, 1024], F32, tag="s",
                                                 name="ps_s", bufs=2)
                                for hp in range(2):
                                    po64 = hp * 64
                                    nc.tensor.matmul(
                                        ps_s[:, hp * 512:(hp + 1) * 512],
                                        lhsT=kT[po64:po64 + 64, pr,
                                                jt * 128:(jt + 1) * 128],
                                        rhs=qT[po64:po64 + 64, pr, isl],
                                        start=True, stop=True)
                                ex = c_sb.tile([128, 1024], BF16, tag="ex",
                                               name="ex", bufs=8)
                                nc.scalar.activation(out=ex, in_=ps_s,
                                                     func=AF.Exp, scale=SCALE)
                                exs[jt] = ex
                            if jt >= DEPTH:
                                o_one(jt - DEPTH, exs.pop(jt - DEPTH))
                            if jt in drip:
                                fn, args = drip.pop(jt)
                                fn(*args)
                        # pair-1 k copy (DVE) after its accumulation
                        if pr == 0:
                            qk1_copy(1)
                        # normalization is deferred into the next chunk's
                        # prologue (one hp right before each po re-zero) so
                        # the boundary dependency chain stays short
                        def mk_norm(hp, _po=po, _ic=ic, _pr=pr):
                            def run():
                                rcp4 = c_sb.tile([128, 4, 1], F32, tag="rcp",
                                                 name="rcp4", bufs=4)
                                nc.vector.reciprocal(out=rcp4,
                                                     in_=_po[hp][:, :, 64:65])
                                dst = o_nat[:, _ic, :, _pr * 2 + hp, :]
                                srcp = _po[hp][:, :, 0:64]
                                rb, _ = broadcast_tensor_aps(rcp4[:, :, :],
                                                             srcp)
                                nc.vector.tensor_tensor(out=dst, in0=srcp,
                                                        in1=rb, op=ALU.mult)
                            return run
                        norm_q.extend([mk_norm(0), mk_norm(1)])
                # tail: flush deferred norms, then the last epilogue
                while norm_q:
                    norm_q.pop(0)()
                for i in range(4):
                    t_one(3, i)
                for i in range(4):
                    d_one(12 + i)

    nc.compile()
    return nc


def kernel(x, gamma, beta, w_qkv, w_out, b_out):
    """Full inputs in, full output out.  Shards batch x head-groups over 8
    cores, runs the SPMD Bass kernel, and sums the partial projections."""
    import ml_dtypes
    bf16 = ml_dtypes.bfloat16

    if "nc" not in _NC_CACHE:
        _NC_CACHE["nc"] = _build()
    nc = _NC_CACHE["nc"]

    x = np.asarray(x, dtype=np.float32)
    gamma = np.asarray(gamma, dtype=np.float32)
    w_qkv = np.asarray(w_qkv, dtype=np.float32)
    w_out = np.asarray(w_out, dtype=np.float32)
    b_out = np.asarray(b_out, dtype=np.float32)

    wg = w_qkv * gamma[:, None]  # fold LN gamma into the QKV projection
    in_maps = []
    for core in range(8):
        b, g = core // 4, core % 4
        cs = slice(g * CI, (g + 1) * CI)
        in_maps.append({
            "x": np.ascontiguousarray(x[b]),
            "wq": np.ascontiguousarray(wg[:, 0 * 1024:1 * 1024][:, cs]).astype(bf16),
            "wk": np.ascontiguousarray(wg[:, 1 * 1024:2 * 1024][:, cs]).astype(bf16),
            "wv": np.ascontiguousarray(wg[:, 2 * 1024:3 * 1024][:, cs]).astype(bf16),
            "wo": np.ascontiguousarray(w_out[cs, :]).astype(bf16),
        })

    res = bass_utils.run_bass_kernel_spmd(nc, in_maps, core_ids=list(range(8)))
    parts = [r["out"] for r in res.results]
    full = np.stack([
        parts[0] + parts[1] + parts[2] + parts[3],
        parts[4] + parts[5] + parts[6] + parts[7],
    ]).astype(np.float32)
    return full + b_out


# revision 31
# speedup vs baseline: 1.0435x; 1.0403x over previous
"""Fused LayerNorm + multi-head attention + out-projection for Trainium2.

Problem: x[2,2048,1024] -> LN -> QKV (16 heads, dh=64) -> softmax attention
-> out proj.  Sharded over 8 NeuronCores as batch(2) x head-groups(4)
(Megatron tensor parallel): each core handles one batch entry and 4 heads,
computing a partial out-projection; the host sums the 4 partials per batch.

Per-core dataflow (T=2048 tokens, D=1024, 4 local heads, dh=64), bf16
matmul inputs everywhere (fp32 psum accumulation):
  A) per 128-token tile: LN via bn_stats/bn_aggr (fp32), xn cast to bf16,
     PE-transpose to persistent xnT [d, tok].
  B) v natural [tok, c] for all 4 heads and qT/kT [c, tok] for head pair 0.
  C) per (head-pair, i-chunk 512): software-pipelined over j-tiles:
     S^T[j,i] paired matmul (two heads on disjoint 64-row PE groups);
     exp((1/8)S) on ACT -> bf16; O[i, d|r] += ex_slice.T @ (V|1) as M=128,
     N=66 bf16 matmuls accumulating in PSUM (denominator r in column 64).
     The scalar-engine exp stream is the wall-clock floor; leftover PE
     work (pair-1 q/k projections during pr=0, O-transposes + output
     projections during pr=1) is dripped one instruction per j-step into
     the PE idle gaps so the tensor engine stays busy enough to hold the
     HAM clock at 2.4 GHz and never stalls the exp stream.
  Normalization is a per-partition scalar multiply (1/r) on DVE in [i, d]
  layout, then PE-transpose back to OT [c, i] for the out-projection.
gamma is folded into w_qkv on the host; beta/b_out are zeros by spec
(b_out still added on the host).
"""
import numpy as np

import concourse.bacc as bacc
import concourse.mybir as mybir
import concourse.tile as tile
from concourse import bass_utils
from concourse.bass import broadcast_tensor_aps
from concourse.masks import make_identity

F32 = mybir.dt.float32
BF16 = mybir.dt.bfloat16
AF = mybir.ActivationFunctionType
ALU = mybir.AluOpType

T = 2048          # tokens per core (one batch entry)
D = 1024          # model dim
HL = 4            # local heads per core
DH = 64           # head dim
CI = HL * DH      # local inner dim = 256
NT = T // 128     # 16 token tiles
NK = D // 128     # 8 dim chunks
LN_EPS = 1e-5
SCALE = DH ** -0.5

_NC_CACHE = {}


def _build():
    nc = bacc.Bacc("TRN2", target_bir_lowering=False, debug=False)

    x = nc.dram_tensor("x", [T, D], F32, kind="ExternalInput")
    wq = nc.dram_tensor("wq", [D, CI], BF16, kind="ExternalInput")
    wk = nc.dram_tensor("wk", [D, CI], BF16, kind="ExternalInput")
    wv = nc.dram_tensor("wv", [D, CI], BF16, kind="ExternalInput")
    wo = nc.dram_tensor("wo", [CI, D], BF16, kind="ExternalInput")
    out = nc.dram_tensor("out", [T, D], F32, kind="ExternalOutput")

    x_t = x.rearrange("(t p) d -> t p d", p=128)          # [16, 128, 1024]
    out_t = out.rearrange("(t p) d -> t p d", p=128)
    wq_t = wq.rearrange("(c p) n -> p c n", p=128)        # [128, 8, 256]
    wk_t = wk.rearrange("(c p) n -> p c n", p=128)
    wv_t = wv.rearrange("(c p) n -> p c n", p=128)
    wo_t = wo.rearrange("(c p) n -> p c n", p=128)        # [128, 2, 1024]

    with tile.TileContext(nc) as tc:
        with tc.tile_pool(name="persist", bufs=1) as persist:
            # constants
            eps = persist.tile([128, 1], F32, name="eps")
            nc.vector.memset(eps, LN_EPS)
            ident_f = persist.tile([128, 128], F32, name="ident_f")
            make_identity(nc, ident_f)
            ident = persist.tile([128, 128], BF16, name="ident")
            nc.vector.tensor_copy(out=ident, in_=ident_f)

            # persistent activations / weights (all bf16)
            xnT = persist.tile([128, NK, T], BF16, name="xnT")  # 32KB/p
            qT = persist.tile([128, 2, T], BF16, name="qT")
            kT = persist.tile([128, 2, T], BF16, name="kT")
            vext = persist.tile([128, NT, HL, 66], BF16, name="vext")
            OT = persist.tile([128, 2, T], BF16, name="OT")
            o_nat = persist.tile([128, 4, 4, HL, 64], BF16, name="o_nat")
            wq_s = persist.tile([128, NK, CI], BF16, name="wq_s")
            wk_s = persist.tile([128, NK, CI], BF16, name="wk_s")
            wv_s = persist.tile([128, NK, CI], BF16, name="wv_s")
            wo_s = persist.tile([128, 2, D], BF16, name="wo_s")

            # ones column (64) of vext; column 65 is zero padding
            nc.vector.memset(vext[:, :, :, 64:65], 1.0)
            nc.vector.memset(vext[:, :, :, 65:66], 0.0)

            # ---- Phase A: LN + transpose; B0: v (all heads) + q/k pair 0
            with (
                tc.tile_pool(name="ab_sb", bufs=3) as ab_sb,
                tc.tile_pool(name="ab_sm", bufs=8) as ab_sm,
                tc.tile_pool(name="ab_ps", bufs=1, space="PSUM") as ab_ps,
            ):
                xts, xns = {}, {}

                def ln_dma(tt):
                    xt = ab_sb.tile([128, D], F32, tag="xt", name="xt", bufs=4)
                    (nc.sync if tt % 2 == 0 else nc.scalar).dma_start(
                        xt, x_t[tt])
                    xts[tt] = xt

                def ln_compute(tt):
                    # stats (DVE) -> rstd, -mu*rstd -> xn on ACT (Identity is
                    # in the sqrt table set, so no extra table load)
                    xt = xts.pop(tt)
                    stats = ab_sm.tile([128, 2, 6], F32, tag="stats",
                                       name="stats")
                    xr = xt.rearrange("p (c f) -> p c f", f=512)
                    for c in range(2):
                        nc.vector.bn_stats(out=stats[:, c, :], in_=xr[:, c, :])
                    mv = ab_sm.tile([128, 2], F32, tag="mv", name="mv")
                    nc.vector.bn_aggr(out=mv, in_=stats)
                    rstd = ab_sm.tile([128, 1], F32, tag="rstd", name="rstd")
                    nc.scalar.activation(out=rstd, in_=mv[:, 1:2], func=AF.Sqrt,
                                         bias=eps, scale=1.0)
                    nc.vector.reciprocal(out=rstd, in_=rstd)
                    nmr = ab_sm.tile([128, 1], F32, tag="nmr", name="nmr")
                    nc.vector.tensor_scalar(out=nmr, in0=mv[:, 0:1],
                                            scalar1=rstd, scalar2=-1.0,
                                            op0=ALU.mult, op1=ALU.mult)
                    xn = ab_sb.tile([128, D], BF16, tag="xn", name="xn")
                    nc.scalar.activation(out=xn, in_=xt, func=AF.Identity,
                                         bias=nmr, scale=rstd)
                    xns[tt] = xn

                def t_tile(tt):
                    # PE-transpose one token tile into xnT
                    xn = xns.pop(tt)
                    for kc4 in range(2):
                        pt = ab_ps.tile([128, 4, 128], BF16, tag="tp",
                                        name="pt", bufs=2)
                        for q in range(4):
                            kc = kc4 * 4 + q
                            nc.tensor.transpose(
                                pt[:, q, :], xn[:, kc * 128:(kc + 1) * 128],
                                ident)
                        nc.vector.tensor_copy(
                            out=xnT[:, kc4 * 4:kc4 * 4 + 4,
                                    tt * 128:(tt + 1) * 128], in_=pt)

                def v_tile(tt):
                    # v natural (all 4 heads) for one token tile
                    pv = ab_ps.tile([128, CI], F32, tag="pv", name="pv",
                                    bufs=2)
                    for kc in range(NK):
                        nc.tensor.matmul(
                            pv,
                            lhsT=xnT[:, kc, tt * 128:(tt + 1) * 128],
                            rhs=wv_s[:, kc, :],
                            start=(kc == 0), stop=(kc == NK - 1))
                    nc.scalar.copy(
                        out=vext[:, tt, :, 0:64],
                        in_=pv.rearrange("p (h d) -> p h d", h=HL))

                def qk_chunk(ic):
                    # qT/kT pair 0 for one token chunk
                    isl = slice(ic * 512, (ic + 1) * 512)
                    pq = ab_ps.tile([128, 2, 512], F32, tag="pq", name="pq",
                                    bufs=2)
                    for kc in range(NK):
                        for pc in range(2):
                            w_src = wq_s if pc == 0 else wk_s
                            nc.tensor.matmul(
                                pq[:, pc, :],
                                lhsT=w_src[:, kc, 0:128],
                                rhs=xnT[:, kc, isl],
                                start=(kc == 0), stop=(kc == NK - 1))
                    nc.scalar.copy(out=qT[:, 0, isl], in_=pq[:, 0, :])
                    nc.scalar.copy(out=kT[:, 0, isl], in_=pq[:, 1, :])

                # staged software pipeline: x DMA runs 6 tiles ahead, the
                # LN chain 2 ahead, the transpose 1 ahead of the v/qk
                # projections, so every engine queue stays fed
                for tt in range(3):
                    ln_dma(tt)
                nc.sync.dma_start(wq_s, wq_t)
                nc.sync.dma_start(wk_s, wk_t)
                nc.sync.dma_start(wv_s, wv_t)
                nc.sync.dma_start(wo_s, wo_t)
                ln_compute(0)
                ln_compute(1)
                t_tile(0)
                for tt in range(NT):
                    v_tile(tt)
                    if tt % 4 == 3:
                        qk_chunk(tt // 4)
                    if tt + 3 < NT:
                        ln_dma(tt + 3)
                    if tt + 2 < NT:
                        ln_compute(tt + 2)
                    if tt + 1 < NT:
                        t_tile(tt + 1)

            # ---------------- Phase C: attention ----------------
            with (
                tc.tile_pool(name="c_sb", bufs=1) as c_sb,
                tc.tile_pool(name="c_ps", bufs=1, space="PSUM") as c_ps,
            ):
                def d_parts(tt):
                    # one out-projection token tile, split into 6 small PE/
                    # DVE tasks (one matmul or copy per j-step drip slot)
                    state = {}

                    def mm(m):
                        ncn, ck = m // 2, m % 2
                        if ck == 0:
                            state[ncn] = c_ps.tile([128, 512], F32, tag="aux",
                                                   name="pd", bufs=2)
                        nc.tensor.matmul(
                            state[ncn],
                            lhsT=OT[:, ck, tt * 128:(tt + 1) * 128],
                            rhs=wo_s[:, ck, ncn * 512:(ncn + 1) * 512],
                            start=(ck == 0), stop=(ck == 1))

                    def cp(ncn):
                        if ncn == 0:
                            state["sb"] = c_sb.tile([128, 1024], F32, tag="ot",
                                                    name="ot_sb", bufs=3)
                        nc.vector.tensor_copy(
                            out=state["sb"][:, ncn * 512:(ncn + 1) * 512],
                            in_=state.pop(ncn))
                        if ncn == 1:
                            nc.sync.dma_start(out_t[tt], state.pop("sb"))

                    return [lambda m=m: mm(m) for m in range(4)] + \
                           [lambda n=n: cp(n) for n in range(2)]

                def t_one(ic, isub):
                    # transpose normalized O [i, c] -> OT [c, i] for one i-sub
                    tp2 = c_ps.tile([128, 2, 128], BF16, tag="aux",
                                    name="tp2", bufs=2)
                    for ck in range(2):
                        nc.tensor.transpose(
                            tp2[:, ck, :],
                            o_nat[:, ic, isub, 2 * ck:2 * ck + 2, :]
                            .rearrange("p a b -> p (a b)"),
                            ident)
                    nc.vector.tensor_copy(
                        out=OT[:, :, ic * 512 + isub * 128:
                               ic * 512 + (isub + 1) * 128],
                        in_=tp2)

                norm_q = []  # deferred per-hp normalizations
                for pr in range(2):
                    for ic in range(4):
                        isl = slice(ic * 512, (ic + 1) * 512)
                        # drip-feed schedule: one small PE task per j-step,
                        # keeping the tensor engine busy under the exp stream
                        drip = {}
                        if pr == 0:
                            # pair-1 q/k projections for this token chunk;
                            # q and k each use a 1-bank aux psum allocation
                            pq1 = {}

                            def qk1(kc, pc, _isl=isl):
                                if kc == 0:
                                    pq1[pc] = c_ps.tile([128, 512], F32,
                                                        tag="aux", name="pq1",
                                                        bufs=2)
                                w_src = wq_s if pc == 0 else wk_s
                                nc.tensor.matmul(
                                    pq1[pc],
                                    lhsT=w_src[:, kc, 128:256],
                                    rhs=xnT[:, kc, _isl],
                                    start=(kc == 0), stop=(kc == NK - 1))

                            def qk1_copy(pc, _isl=isl):
                                dst = qT if pc == 0 else kT
                                nc.vector.tensor_copy(out=dst[:, 1, _isl],
                                                      in_=pq1.pop(pc))

                            for kc in range(NK):
                                drip[kc] = [(qk1, (kc, 0))]
                                drip[NK + kc] = [(qk1, (kc, 1))]
                            drip[NK + NK] = [(qk1_copy, (0,))]
                        else:
                            # epilogue of chunk ic-1: transposes + projection,
                            # at most ~one matmul-equivalent per j-step
                            if ic > 0:
                                for i in range(4):
                                    drip.setdefault(4 * i + 1, []).append(
                                        (t_one, (ic - 1, i)))
                                    parts = d_parts((ic - 1) * 4 + i)
                                    for m, fn in enumerate(parts):
                                        drip.setdefault(4 * i + 1 + m, []) \
                                            .append((fn, ()))

                        po = [c_ps.tile([128, 4, 128], F32, tag=f"o{j}",
                                        name=f"po{j}", bufs=1)
                              for j in range(2)]
                        if norm_q:
                            norm_q.pop(0)()     # norm prev hp0 (frees o0)
                        nc.vector.memset(po[0], 0.0)
                        if norm_q:
                            norm_q.pop(0)()     # norm prev hp1 (frees o1)
                        nc.vector.memset(po[1], 0.0)

                        def o_one(jt, exv):
                            for hp in range(2):
                                for isub in range(4):
                                    nc.tensor.matmul(
                                        po[hp][:, isub, 0:66],
                                        lhsT=exv[:, hp * 512 + isub * 128:
                                                 hp * 512 + (isub + 1) * 128],
                                        rhs=vext[:, jt, pr * 2 + hp, :],
                                        start=False, stop=(jt == NT - 1),
                                        skip_group_check=True)

                        # software-pipelined: issue S(jt)+exp(jt) three steps
                        # ahead of O(jt-3) so the in-order PE queue never
                        # blocks the exp stream behind the O accumulation's
                        # chunk-boundary dependency (normalize + re-zero)
                        DEPTH = 1
                        exs = {}
                        for jt in range(NT + DEPTH):
                            if jt < NT:
                                ps_s = c_ps.tile([128, 1024], F32, tag="s",
                                                 name="ps_s", bufs=2)
                                for hp in range(2):
                                    po64 = hp * 64
                                    nc.tensor.matmul(
                                        ps_s[:, hp * 512:(hp + 1) * 512],
                                        lhsT=kT[po64:po64 + 64, pr,
                                                jt * 128:(jt + 1) * 128],
                                        rhs=qT[po64:po64 + 64, pr, isl],
                                        start=True, stop=True)
                                ex = c_sb.tile([128, 1024], BF16, tag="ex",
                                               name="ex", bufs=8)
                                nc.scalar.activation(out=ex, in_=ps_s,
                                                     func=AF.Exp, scale=SCALE)
                                exs[jt] = ex
                            if jt >= DEPTH:
                                o_one(jt - DEPTH, exs.pop(jt - DEPTH))
                            for fn, args in drip.pop(jt, ()):
                                fn(*args)
                        # flush drip tasks scheduled past the last j-step
                        for slot in sorted(drip):
                            for fn, args in drip.pop(slot):
                                fn(*args)
                        # pair-1 k copy (DVE) after its accumulation
                        if pr == 0:
                            qk1_copy(1)
                        # normalization is deferred into the next chunk's
                        # prologue (one hp right before each po re-zero) so
                        # the boundary dependency chain stays short
                        def mk_norm(hp, _po=po, _ic=ic, _pr=pr):
                            def run():
                                rcp4 = c_sb.tile([128, 4, 1], F32, tag="rcp",
                                                 name="rcp4", bufs=4)
                                nc.vector.reciprocal(out=rcp4,
                                                     in_=_po[hp][:, :, 64:65])
                                dst = o_nat[:, _ic, :, _pr * 2 + hp, :]
                                srcp = _po[hp][:, :, 0:64]
                                rb, _ = broadcast_tensor_aps(rcp4[:, :, :],
                                                             srcp)
                                nc.vector.tensor_tensor(out=dst, in0=srcp,
                                                        in1=rb, op=ALU.mult)
                            return run
                        norm_q.extend([mk_norm(0), mk_norm(1)])
                # tail: flush deferred norms, then the last epilogue
                while norm_q:
                    norm_q.pop(0)()
                for i in range(4):
                    t_one(3, i)
                for i in range(4):
                    for fn in d_parts(12 + i):
                        fn()

    nc.compile()
    return nc


def kernel(x, gamma, beta, w_qkv, w_out, b_out):
    """Full inputs in, full output out.  Shards batch x head-groups over 8
    cores, runs the SPMD Bass kernel, and sums the partial projections."""
    import ml_dtypes
    bf16 = ml_dtypes.bfloat16

    if "nc" not in _NC_CACHE:
        _NC_CACHE["nc"] = _build()
    nc = _NC_CACHE["nc"]

    x = np.asarray(x, dtype=np.float32)
    gamma = np.asarray(gamma, dtype=np.float32)
    w_qkv = np.asarray(w_qkv, dtype=np.float32)
    w_out = np.asarray(w_out, dtype=np.float32)
    b_out = np.asarray(b_out, dtype=np.float32)

    wg = w_qkv * gamma[:, None]  # fold LN gamma into the QKV projection
    in_maps = []
    for core in range(8):
        b, g = core // 4, core % 4
        cs = slice(g * CI, (g + 1) * CI)
        in_maps.append({
            "x": np.ascontiguousarray(x[b]),
            "wq": np.ascontiguousarray(wg[:, 0 * 1024:1 * 1024][:, cs]).astype(bf16),
            "wk": np.ascontiguousarray(wg[:, 1 * 1024:2 * 1024][:, cs]).astype(bf16),
            "wv": np.ascontiguousarray(wg[:, 2 * 1024:3 * 1024][:, cs]).astype(bf16),
            "wo": np.ascontiguousarray(w_out[cs, :]).astype(bf16),
        })

    res = bass_utils.run_bass_kernel_spmd(nc, in_maps, core_ids=list(range(8)))
    parts = [r["out"] for r in res.results]
    full = np.stack([
        parts[0] + parts[1] + parts[2] + parts[3],
        parts[4] + parts[5] + parts[6] + parts[7],
    ]).astype(np.float32)
    return full + b_out


# revision 33
# speedup vs baseline: 1.2358x; 1.1843x over previous
"""Fused LayerNorm + multi-head attention + out-projection for Trainium2.

Problem: x[2,2048,1024] -> LN -> QKV (16 heads, dh=64) -> softmax attention
-> out proj.  Sharded over 8 NeuronCores as batch(2) x head-groups(4)
(Megatron tensor parallel): each core handles one batch entry and 4 heads,
computing a partial out-projection; the host sums the 4 partials per batch.

Per-core dataflow (T=2048 tokens, D=1024, 4 local heads, dh=64), bf16
matmul inputs everywhere (fp32 psum accumulation):
  A) per 128-token tile: LN via bn_stats/bn_aggr (fp32), xn cast to bf16,
     PE-transpose to persistent xnT [d, tok].
  B) v natural [tok, c] for all 4 heads and qT/kT [c, tok] for head pair 0.
  C) per (head-pair, i-chunk 512): software-pipelined over j-tiles:
     S^T[j,i] paired matmul (two heads on disjoint 64-row PE groups);
     exp((1/8)S) on ACT -> bf16; O[i, d|r] += ex_slice.T @ (V|1) as M=128,
     N=66 bf16 matmuls accumulating in PSUM (denominator r in column 64).
     The scalar-engine exp stream is the wall-clock floor; leftover PE
     work (pair-1 q/k projections during pr=0, O-transposes + output
     projections during pr=1) is dripped one instruction per j-step into
     the PE idle gaps so the tensor engine stays busy enough to hold the
     HAM clock at 2.4 GHz and never stalls the exp stream.
  Normalization is a per-partition scalar multiply (1/r) on DVE in [i, d]
  layout, then PE-transpose back to OT [c, i] for the out-projection.
gamma is folded into w_qkv on the host; beta/b_out are zeros by spec
(b_out still added on the host).
"""
import numpy as np

import concourse.bacc as bacc
import concourse.mybir as mybir
import concourse.tile as tile
from concourse import bass_utils
from concourse.bass import broadcast_tensor_aps
from concourse.masks import make_identity

F32 = mybir.dt.float32
BF16 = mybir.dt.bfloat16
AF = mybir.ActivationFunctionType
ALU = mybir.AluOpType

T = 2048          # tokens per core (one batch entry)
D = 1024          # model dim
HL = 4            # local heads per core
DH = 64           # head dim
CI = HL * DH      # local inner dim = 256
NT = T // 128     # 16 token tiles
NK = D // 128     # 8 dim chunks
LN_EPS = 1e-5
SCALE = DH ** -0.5

_NC_CACHE = {}


def _build():
    nc = bacc.Bacc("TRN2", target_bir_lowering=False, debug=False)

    x = nc.dram_tensor("x", [T, D], BF16, kind="ExternalInput")
    wq = nc.dram_tensor("wq", [D, CI], BF16, kind="ExternalInput")
    wk = nc.dram_tensor("wk", [D, CI], BF16, kind="ExternalInput")
    wv = nc.dram_tensor("wv", [D, CI], BF16, kind="ExternalInput")
    wo = nc.dram_tensor("wo", [CI, D], BF16, kind="ExternalInput")
    out = nc.dram_tensor("out", [T, D], F32, kind="ExternalOutput")

    x_t = x.rearrange("(t p) d -> t p d", p=128)          # [16, 128, 1024]
    out_t = out.rearrange("(t p) d -> t p d", p=128)
    wq_t = wq.rearrange("(c p) n -> p c n", p=128)        # [128, 8, 256]
    wk_t = wk.rearrange("(c p) n -> p c n", p=128)
    wv_t = wv.rearrange("(c p) n -> p c n", p=128)
    wo_t = wo.rearrange("(c p) n -> p c n", p=128)        # [128, 2, 1024]

    with tile.TileContext(nc) as tc:
        with tc.tile_pool(name="persist", bufs=1) as persist:
            # constants
            eps = persist.tile([128, 1], F32, name="eps")
            nc.vector.memset(eps, LN_EPS)
            ident_f = persist.tile([128, 128], F32, name="ident_f")
            make_identity(nc, ident_f)
            ident = persist.tile([128, 128], BF16, name="ident")
            nc.vector.tensor_copy(out=ident, in_=ident_f)

            # persistent activations / weights (all bf16)
            xnT = persist.tile([128, NK, T], BF16, name="xnT")  # 32KB/p
            qT = persist.tile([128, 2, T], BF16, name="qT")
            kT = persist.tile([128, 2, T], BF16, name="kT")
            vext = persist.tile([128, NT, HL, 66], BF16, name="vext")
            OT = persist.tile([128, 2, T], BF16, name="OT")
            o_nat = persist.tile([128, 4, 4, HL, 64], BF16, name="o_nat")
            wq_s = persist.tile([128, NK, CI], BF16, name="wq_s")
            wk_s = persist.tile([128, NK, CI], BF16, name="wk_s")
            wv_s = persist.tile([128, NK, CI], BF16, name="wv_s")
            wo_s = persist.tile([128, 2, D], BF16, name="wo_s")

            # ones column (64) of vext; column 65 is zero padding
            nc.vector.memset(vext[:, :, :, 64:65], 1.0)
            nc.vector.memset(vext[:, :, :, 65:66], 0.0)

            # ---- Phase A: LN + transpose; B0: v (all heads) + q/k pair 0
            with (
                tc.tile_pool(name="ab_sb", bufs=3) as ab_sb,
                tc.tile_pool(name="ab_sm", bufs=8) as ab_sm,
                tc.tile_pool(name="ab_ps", bufs=1, space="PSUM") as ab_ps,
            ):
                xts, xns = {}, {}

                def ln_dma(tt):
                    xt = ab_sb.tile([128, D], BF16, tag="xt", name="xt",
                                    bufs=4)
                    (nc.sync if tt % 2 == 0 else nc.scalar).dma_start(
                        xt, x_t[tt])
                    xts[tt] = xt

                def ln_compute(tt):
                    # stats (DVE) -> rstd, -mu*rstd -> xn on ACT (Identity is
                    # in the sqrt table set, so no extra table load)
                    xt = xts.pop(tt)
                    stats = ab_sm.tile([128, 2, 6], F32, tag="stats",
                                       name="stats")
                    xr = xt.rearrange("p (c f) -> p c f", f=512)
                    for c in range(2):
                        nc.vector.bn_stats(out=stats[:, c, :], in_=xr[:, c, :])
                    mv = ab_sm.tile([128, 2], F32, tag="mv", name="mv")
                    nc.vector.bn_aggr(out=mv, in_=stats)
                    rstd = ab_sm.tile([128, 1], F32, tag="rstd", name="rstd")
                    nc.scalar.activation(out=rstd, in_=mv[:, 1:2], func=AF.Sqrt,
                                         bias=eps, scale=1.0)
                    nc.vector.reciprocal(out=rstd, in_=rstd)
                    nmr = ab_sm.tile([128, 1], F32, tag="nmr", name="nmr")
                    nc.vector.tensor_scalar(out=nmr, in0=mv[:, 0:1],
                                            scalar1=rstd, scalar2=-1.0,
                                            op0=ALU.mult, op1=ALU.mult)
                    xn = ab_sb.tile([128, D], BF16, tag="xn", name="xn")
                    nc.scalar.activation(out=xn, in_=xt, func=AF.Identity,
                                         bias=nmr, scale=rstd)
                    xns[tt] = xn

                def t_tile(tt):
                    # PE-transpose one token tile into xnT
                    xn = xns.pop(tt)
                    for kc4 in range(2):
                        pt = ab_ps.tile([128, 4, 128], BF16, tag="tp",
                                        name="pt", bufs=2)
                        for q in range(4):
                            kc = kc4 * 4 + q
                            nc.tensor.transpose(
                                pt[:, q, :], xn[:, kc * 128:(kc + 1) * 128],
                                ident)
                        if kc4 == 0:
                            nc.vector.tensor_copy(
                                out=xnT[:, 0:4,
                                        tt * 128:(tt + 1) * 128], in_=pt)
                        else:
                            nc.scalar.copy(
                                out=xnT[:, 4:8,
                                        tt * 128:(tt + 1) * 128], in_=pt)

                def v_tile(tt):
                    # v natural (all 4 heads) for one token tile
                    pv = ab_ps.tile([128, CI], F32, tag="pv", name="pv",
                                    bufs=2)
                    for kc in range(NK):
                        nc.tensor.matmul(
                            pv,
                            lhsT=xnT[:, kc, tt * 128:(tt + 1) * 128],
                            rhs=wv_s[:, kc, :],
                            start=(kc == 0), stop=(kc == NK - 1))
                    nc.scalar.copy(
                        out=vext[:, tt, :, 0:64],
                        in_=pv.rearrange("p (h d) -> p h d", h=HL))

                def qk_chunk(ic):
                    # qT/kT pair 0 for one token chunk
                    isl = slice(ic * 512, (ic + 1) * 512)
                    pq = ab_ps.tile([128, 2, 512], F32, tag="pq", name="pq",
                                    bufs=2)
                    for kc in range(NK):
                        for pc in range(2):
                            w_src = wq_s if pc == 0 else wk_s
                            nc.tensor.matmul(
                                pq[:, pc, :],
                                lhsT=w_src[:, kc, 0:128],
                                rhs=xnT[:, kc, isl],
                                start=(kc == 0), stop=(kc == NK - 1))
                    nc.scalar.copy(out=qT[:, 0, isl], in_=pq[:, 0, :])
                    nc.scalar.copy(out=kT[:, 0, isl], in_=pq[:, 1, :])

                # staged software pipeline: x DMA runs 6 tiles ahead, the
                # LN chain 2 ahead, the transpose 1 ahead of the v/qk
                # projections, so every engine queue stays fed
                for tt in range(3):
                    ln_dma(tt)
                nc.sync.dma_start(wq_s, wq_t)
                nc.sync.dma_start(wk_s, wk_t)
                nc.sync.dma_start(wv_s, wv_t)
                nc.sync.dma_start(wo_s, wo_t)
                ln_compute(0)
                ln_compute(1)
                t_tile(0)
                for tt in range(NT):
                    v_tile(tt)
                    if tt % 4 == 3:
                        qk_chunk(tt // 4)
                    if tt + 3 < NT:
                        ln_dma(tt + 3)
                    if tt + 2 < NT:
                        ln_compute(tt + 2)
                    if tt + 1 < NT:
                        t_tile(tt + 1)

            # ---------------- Phase C: attention ----------------
            with (
                tc.tile_pool(name="c_sb", bufs=1) as c_sb,
                tc.tile_pool(name="c_ps", bufs=1, space="PSUM") as c_ps,
            ):
                def d_parts(tt):
                    # one out-projection token tile, split into 6 small PE/
                    # DVE tasks (one matmul or copy per j-step drip slot)
                    state = {}

                    def mm(m):
                        ncn, ck = m // 2, m % 2
                        if ck == 0:
                            state[ncn] = c_ps.tile([128, 512], F32, tag="aux",
                                                   name="pd", bufs=2)
                        nc.tensor.matmul(
                            state[ncn],
                            lhsT=OT[:, ck, tt * 128:(tt + 1) * 128],
                            rhs=wo_s[:, ck, ncn * 512:(ncn + 1) * 512],
                            start=(ck == 0), stop=(ck == 1))

                    def cp(ncn):
                        if ncn == 0:
                            state["sb"] = c_sb.tile([128, 1024], F32, tag="ot",
                                                    name="ot_sb", bufs=3)
                        nc.vector.tensor_copy(
                            out=state["sb"][:, ncn * 512:(ncn + 1) * 512],
                            in_=state.pop(ncn))
                        if ncn == 1:
                            nc.sync.dma_start(out_t[tt], state.pop("sb"))

                    return [lambda m=m: mm(m) for m in range(4)] + \
                           [lambda n=n: cp(n) for n in range(2)]

                def t_one(ic, isub):
                    # transpose normalized O [i, c] -> OT [c, i] for one i-sub
                    tp2 = c_ps.tile([128, 2, 128], BF16, tag="aux",
                                    name="tp2", bufs=2)
                    for ck in range(2):
                        nc.tensor.transpose(
                            tp2[:, ck, :],
                            o_nat[:, ic, isub, 2 * ck:2 * ck + 2, :]
                            .rearrange("p a b -> p (a b)"),
                            ident)
                    nc.vector.tensor_copy(
                        out=OT[:, :, ic * 512 + isub * 128:
                               ic * 512 + (isub + 1) * 128],
                        in_=tp2)

                norm_q = []  # deferred per-hp normalizations
                for pr in range(2):
                    for ic in range(4):
                        isl = slice(ic * 512, (ic + 1) * 512)
                        # drip-feed schedule: one small PE task per j-step,
                        # keeping the tensor engine busy under the exp stream
                        drip = {}
                        if pr == 0:
                            # pair-1 q/k projections for this token chunk;
                            # q and k each use a 1-bank aux psum allocation
                            pq1 = {}

                            def qk1(kc, pc, _isl=isl):
                                if kc == 0:
                                    pq1[pc] = c_ps.tile([128, 512], F32,
                                                        tag="aux", name="pq1",
                                                        bufs=2)
                                w_src = wq_s if pc == 0 else wk_s
                                nc.tensor.matmul(
                                    pq1[pc],
                                    lhsT=w_src[:, kc, 128:256],
                                    rhs=xnT[:, kc, _isl],
                                    start=(kc == 0), stop=(kc == NK - 1))

                            def qk1_copy(pc, _isl=isl):
                                dst = qT if pc == 0 else kT
                                nc.vector.tensor_copy(out=dst[:, 1, _isl],
                                                      in_=pq1.pop(pc))

                            for kc in range(NK):
                                drip[kc] = [(qk1, (kc, 0))]
                                drip[NK + kc] = [(qk1, (kc, 1))]
                            drip[NK + NK] = [(qk1_copy, (0,))]
                        else:
                            # epilogue of chunk ic-1: transposes + projection,
                            # at most ~one matmul-equivalent per j-step
                            if ic > 0:
                                for i in range(4):
                                    drip.setdefault(4 * i + 1, []).append(
                                        (t_one, (ic - 1, i)))
                                    parts = d_parts((ic - 1) * 4 + i)
                                    for m, fn in enumerate(parts):
                                        drip.setdefault(4 * i + 1 + m, []) \
                                            .append((fn, ()))

                        po = [c_ps.tile([128, 4, 128], F32, tag=f"o{j}",
                                        name=f"po{j}", bufs=1)
                              for j in range(2)]
                        if norm_q:
                            norm_q.pop(0)()     # norm prev hp0 (frees o0)
                        nc.vector.memset(po[0], 0.0)
                        if norm_q:
                            norm_q.pop(0)()     # norm prev hp1 (frees o1)
                        nc.vector.memset(po[1], 0.0)

                        def o_one(jt, exv):
                            for hp in range(2):
                                for isub in range(4):
                                    nc.tensor.matmul(
                                        po[hp][:, isub, 0:66],
                                        lhsT=exv[:, hp * 512 + isub * 128:
                                                 hp * 512 + (isub + 1) * 128],
                                        rhs=vext[:, jt, pr * 2 + hp, :],
                                        start=False, stop=(jt == NT - 1),
                                        skip_group_check=True)

                        # software-pipelined: issue S(jt)+exp(jt) three steps
                        # ahead of O(jt-3) so the in-order PE queue never
                        # blocks the exp stream behind the O accumulation's
                        # chunk-boundary dependency (normalize + re-zero)
                        DEPTH = 1
                        exs = {}
                        for jt in range(NT + DEPTH):
                            if jt < NT:
                                ps_s = c_ps.tile([128, 1024], F32, tag="s",
                                                 name="ps_s", bufs=2)
                                for hp in range(2):
                                    po64 = hp * 64
                                    nc.tensor.matmul(
                                        ps_s[:, hp * 512:(hp + 1) * 512],
                                        lhsT=kT[po64:po64 + 64, pr,
                                                jt * 128:(jt + 1) * 128],
                                        rhs=qT[po64:po64 + 64, pr, isl],
                                        start=True, stop=True)
                                ex = c_sb.tile([128, 1024], BF16, tag="ex",
                                               name="ex", bufs=8)
                                nc.scalar.activation(out=ex, in_=ps_s,
                                                     func=AF.Exp, scale=SCALE)
                                exs[jt] = ex
                            if jt >= DEPTH:
                                o_one(jt - DEPTH, exs.pop(jt - DEPTH))
                            for fn, args in drip.pop(jt, ()):
                                fn(*args)
                        # flush drip tasks scheduled past the last j-step
                        for slot in sorted(drip):
                            for fn, args in drip.pop(slot):
                                fn(*args)
                        # pair-1 k copy (DVE) after its accumulation
                        if pr == 0:
                            qk1_copy(1)
                        # normalization is deferred into the next chunk's
                        # prologue (one hp right before each po re-zero) so
                        # the boundary dependency chain stays short
                        def mk_norm(hp, _po=po, _ic=ic, _pr=pr):
                            def run():
                                rcp4 = c_sb.tile([128, 4, 1], F32, tag="rcp",
                                                 name="rcp4", bufs=4)
                                nc.vector.reciprocal(out=rcp4,
                                                     in_=_po[hp][:, :, 64:65])
                                dst = o_nat[:, _ic, :, _pr * 2 + hp, :]
                                srcp = _po[hp][:, :, 0:64]
                                rb, _ = broadcast_tensor_aps(rcp4[:, :, :],
                                                             srcp)
                                nc.vector.tensor_tensor(out=dst, in0=srcp,
                                                        in1=rb, op=ALU.mult)
                            return run
                        norm_q.extend([mk_norm(0), mk_norm(1)])
                # tail: flush deferred norms, then the last epilogue
                while norm_q:
                    norm_q.pop(0)()
                for i in range(4):
                    t_one(3, i)
                for i in range(4):
                    for fn in d_parts(12 + i):
                        fn()

    nc.compile()
    return nc


def kernel(x, gamma, beta, w_qkv, w_out, b_out):
    """Full inputs in, full output out.  Shards batch x head-groups over 8
    cores, runs the SPMD Bass kernel, and sums the partial projections."""
    import ml_dtypes
    bf16 = ml_dtypes.bfloat16

    if "nc" not in _NC_CACHE:
        _NC_CACHE["nc"] = _build()
    nc = _NC_CACHE["nc"]

    x = np.asarray(x, dtype=np.float32)
    gamma = np.asarray(gamma, dtype=np.float32)
    w_qkv = np.asarray(w_qkv, dtype=np.float32)
    w_out = np.asarray(w_out, dtype=np.float32)
    b_out = np.asarray(b_out, dtype=np.float32)

    wg = w_qkv * gamma[:, None]  # fold LN gamma into the QKV projection
    in_maps = []
    for core in range(8):
        b, g = core // 4, core % 4
        cs = slice(g * CI, (g + 1) * CI)
        in_maps.append({
            "x": np.ascontiguousarray(x[b]).astype(bf16),
            "wq": np.ascontiguousarray(wg[:, 0 * 1024:1 * 1024][:, cs]).astype(bf16),
            "wk": np.ascontiguousarray(wg[:, 1 * 1024:2 * 1024][:, cs]).astype(bf16),
            "wv": np.ascontiguousarray(wg[:, 2 * 1024:3 * 1024][:, cs]).astype(bf16),
            "wo": np.ascontiguousarray(w_out[cs, :]).astype(bf16),
        })

    res = bass_utils.run_bass_kernel_spmd(nc, in_maps, core_ids=list(range(8)))
    parts = [r["out"] for r in res.results]
    full = np.stack([
        parts[0] + parts[1] + parts[2] + parts[3],
        parts[4] + parts[5] + parts[6] + parts[7],
    ]).astype(np.float32)
    return full + b_out
